# revision 25
# baseline (speedup 1.0000x reference)
"""Distributed Trainium2 kernel for nn_AncProbsLayer.

Math (reference):
    tau[b,h]  = softplus(tau_kernel[h, rate_indices[b,h]])
    R,p,Q     from tiny (H,K,20,20) kernels; Sm = D^1/2 Q D^-1/2; lam,U = eigh(Sm)
    P[b,h,k]  = D^-1/2 U diag(exp(tau*lam)) U^T D^1/2
    out       = einsum('blhz,bhkzs->blhks', inputs, P)

Device algorithm (V,W tiny host-precomputed eigen matrices; E from a
device-side indirect-DMA gather of tau_kernel + softplus + exp):
    P_comb[b]  = BDV @ (diag(E[b]) @ BDW)          (40x80, per-batch stationary)
    out[b,l,:] = in[b,l,:] @ P_comb[b]             (one matmul per batch)
The big matmul runs as a 3-term bf16 hi/lo split (hi@hi + lo@hi + hi@lo,
fp32 PSUM accumulation) — bf16 products are exact in fp32, so accuracy is
~fp32 while the PE streams at 1 cycle/row instead of fp32's 4.

Distribution: data-parallel over batch B across 8 cores (128 b each);
tiny kernels + tau table replicated. Streams are fed feature-major
([40, rows]) so the PE contracts over the partition dim.
"""

import numpy as np
import ml_dtypes

import concourse.bass as bass
import concourse.bacc as bacc
import concourse.mybir as mybir
from concourse.tile import TileContext
from concourse.masks import make_identity
from concourse.bass_utils import run_bass_kernel_spmd

# Problem constants (hardcoded per the harness contract)
B, L, H, K, S = 1024, 512, 2, 2, 20
NUM_RATES = 100000
NCORES = 8
BPC = B // NCORES          # 128 batches per core
ROWS = BPC * L             # 65536 stream rows per core
HZ = H * S                 # 40  (input feature dim)
HKS = H * K * S            # 80  (output feature dim)
CB = 8                     # batches per DMA chunk
F32 = mybir.dt.float32
BF16 = mybir.dt.bfloat16
NPBF16 = np.dtype(ml_dtypes.bfloat16)

_NC_CACHE = {}


def build_nc():
    nc = bacc.Bacc(
        "TRN2", target_bir_lowering=False, debug=False, num_devices=NCORES
    )
    # input split by batch parity: even b -> SBUF partitions 0-39, odd b ->
    # partitions 64-103, so paired matmuls run on disjoint PE row groups
    # concurrently and input DMA covers both SBUF port halves.
    in_hi_e = nc.declare_dram_parameter("in_hi_e", [HZ, ROWS // 2], BF16, isOutput=False)
    in_hi_o = nc.declare_dram_parameter("in_hi_o", [HZ, ROWS // 2], BF16, isOutput=False)
    in_lo_e = nc.declare_dram_parameter("in_lo_e", [HZ, ROWS // 2], BF16, isOutput=False)
    in_lo_o = nc.declare_dram_parameter("in_lo_o", [HZ, ROWS // 2], BF16, isOutput=False)
    tau_tab = nc.declare_dram_parameter("tau_tab", [H * NUM_RATES, 1], F32, isOutput=False)
    offs = nc.declare_dram_parameter("offs", [BPC, H], mybir.dt.int32, isOutput=False)
    bdvT = nc.declare_dram_parameter("bdvT", [HKS, HZ], F32, isOutput=False)
    bdw = nc.declare_dram_parameter("bdw", [HKS, HKS], F32, isOutput=False)
    lam_rep = nc.declare_dram_parameter("lam_rep", [BPC, HKS], F32, isOutput=False)
    out = nc.declare_dram_parameter("out", [HKS, ROWS], F32, isOutput=True)

    QB = 32                    # batches per pc quarter
    NQ = BPC // QB             # 4 quarters
    with TileContext(nc) as tc:
        with (
            tc.tile_pool(name="const", bufs=1) as cpool,
            tc.tile_pool(name="setup", bufs=2) as spool,
            tc.tile_pool(name="inp", bufs=4) as ipool,
            tc.tile_pool(name="ost", bufs=4) as opool,
            tc.tile_pool(name="psE", bufs=1, space="PSUM") as psE,
            tc.tile_pool(name="psP", bufs=2, space="PSUM") as psP,
            tc.tile_pool(name="psO", bufs=2, space="PSUM") as psO,
        ):
            # ---- constants / setup ----
            bdvT_t = cpool.tile([HKS, HZ], dtype=F32)
            nc.sync.dma_start(out=bdvT_t[:], in_=bdvT[:])
            bdw_t = cpool.tile([HKS, HKS], dtype=F32)
            nc.sync.dma_start(out=bdw_t[:], in_=bdw[:])
            lam_t = cpool.tile([BPC, HKS], dtype=F32)
            nc.sync.dma_start(out=lam_t[:], in_=lam_rep[:])
            offs_t = cpool.tile([BPC, H], dtype=mybir.dt.int32)
            nc.sync.dma_start(out=offs_t[:], in_=offs[:])
            ident = cpool.tile([BPC, BPC], dtype=F32)
            make_identity(nc, ident[:])

            # ---- gather tau values: tau_raw[b,h] = tau_tab[offs[b,h]] ----
            tau_raw = cpool.tile([BPC, H], dtype=F32)
            for h in range(H):
                nc.gpsimd.indirect_dma_start(
                    out=tau_raw[:, h : h + 1],
                    out_offset=None,
                    in_=tau_tab[:],
                    in_offset=bass.IndirectOffsetOnAxis(
                        ap=offs_t[:, h : h + 1], axis=0
                    ),
                )
            # softplus(x) = ln(exp(x) + 1): the ACT table set
            # (natural_log_exp_and_others) has exp/ln/copy but no softplus.
            tau_ex = cpool.tile([BPC, H], dtype=F32)
            nc.scalar.activation(
                tau_ex[:], tau_raw[:], mybir.ActivationFunctionType.Exp
            )
            tau_sp = cpool.tile([BPC, H], dtype=F32)
            nc.scalar.activation(
                tau_sp[:], tau_ex[:], mybir.ActivationFunctionType.Ln, bias=1.0
            )

            # ---- E[b, hks] = exp(tau[b,h] * lam[hks]) ----
            E = cpool.tile([BPC, HKS], dtype=F32)
            for h in range(H):
                sl = slice(h * K * S, (h + 1) * K * S)
                nc.scalar.activation(
                    E[:, sl],
                    lam_t[:, sl],
                    mybir.ActivationFunctionType.Exp,
                    scale=tau_sp[:, h : h + 1],
                )
            # transpose E -> E_T [80, 128] so per-b columns are per-partition scalars
            e_ps = psE.tile([HKS, BPC], dtype=F32, space="PSUM")
            nc.tensor.transpose(out=e_ps[:], in_=E[:], identity=ident[:])
            e_t = cpool.tile([HKS, BPC], dtype=F32)
            nc.vector.tensor_copy(out=e_t[:], in_=e_ps[:])

            # ---- setup phase: P_comb hi/lo stationaries, in 4 quarters so
            # the main loop can start once the first quarter is ready.
            # bdwe[:, b*80+j] = BDW[:, j] * E_T[:, b] via stride-0 broadcast
            # APs; P_comb = BDV @ bdwe in batched fp32 matmuls (n=512).
            pc_hi_q, pc_lo_q = [], []
            for q in range(NQ):
                bdwe = spool.tile([HKS, QB * HKS], dtype=F32, tag="bdwe")
                nc.vector.tensor_mul(
                    bdwe[:].rearrange("p (b j) -> p b j", j=HKS),
                    bdw_t[:, None, :].to_broadcast([HKS, QB, HKS]),
                    e_t[:, q * QB : (q + 1) * QB].to_broadcast([HKS, QB, HKS]),
                )
                hi_t = cpool.tile([128, QB * HKS], dtype=BF16, tag=f"pch{q}")
                lo_t = cpool.tile([128, QB * HKS], dtype=BF16, tag=f"pcl{q}")
                for m in range((QB * HKS) // L):
                    cs = slice(m * L, (m + 1) * L)
                    pc_ps = psP.tile([HZ, L], dtype=F32, space="PSUM", tag="pc")
                    nc.tensor.matmul(
                        pc_ps[:], lhsT=bdvT_t[:], rhs=bdwe[:, cs],
                        start=True, stop=True,
                    )
                    nc.scalar.copy(out=hi_t[:HZ, cs], in_=pc_ps[:])
                    nc.vector.tensor_sub(lo_t[:HZ, cs], pc_ps[:], hi_t[:HZ, cs])
                # replicate the quarter to partitions 64-103 for the odd-b
                # row-group (SBUF->SBUF DMA crosses partitions; SWDGE ring
                # keeps the SP/ACT HWDGE rings free)
                nc.gpsimd.dma_start(out=hi_t[64 : 64 + HZ, :], in_=hi_t[:HZ, :])
                nc.gpsimd.dma_start(out=lo_t[64 : 64 + HZ, :], in_=lo_t[:HZ, :])
                pc_hi_q.append(hi_t)
                pc_lo_q.append(lo_t)

            # ---- main stream: 16 chunks x 8 batches (4 even/odd pairs) ----
            HP = 2 * L  # columns per parity stream per chunk... CB/2 pairs
            for ci in range(BPC // CB):
                csl = slice(ci * (CB // 2) * L, (ci + 1) * (CB // 2) * L)
                it_hi = ipool.tile([128, (CB // 2) * L], dtype=BF16, tag="it_hi")
                nc.sync.dma_start(out=it_hi[:HZ, :], in_=in_hi_e[:, csl])
                nc.sync.dma_start(out=it_hi[64 : 64 + HZ, :], in_=in_hi_o[:, csl])
                it_lo = ipool.tile([128, (CB // 2) * L], dtype=BF16, tag="it_lo")
                nc.sync.dma_start(out=it_lo[:HZ, :], in_=in_lo_e[:, csl])
                nc.sync.dma_start(out=it_lo[64 : 64 + HZ, :], in_=in_lo_o[:, csl])
                for jj in range(CB // 2):
                    # even/odd batch pair: even on PE rows 0-39, odd on rows
                    # 64-103 (disjoint row groups run concurrently); the two
                    # accumulation groups land in the 2 banks of one PSUM tile
                    be = ci * CB + jj * 2
                    bo = be + 1
                    qe, bqe = be // QB, be % QB
                    qo, bqo = bo // QB, bo % QB
                    pse = slice(bqe * HKS, (bqe + 1) * HKS)
                    pso = slice(bqo * HKS, (bqo + 1) * HKS)
                    xs = slice(jj * L, (jj + 1) * L)
                    o_ps = psO.tile([HKS, 2 * L], dtype=F32, space="PSUM", tag="o")
                    oe = slice(0, L)
                    oo = slice(L, 2 * L)
                    HI = slice(64, 64 + HZ)
                    LO = slice(0, HZ)
                    nc.tensor.matmul(
                        o_ps[:, oe], lhsT=pc_hi_q[qe][LO, pse], rhs=it_hi[LO, xs],
                        start=True, stop=False,
                    )
                    nc.tensor.matmul(
                        o_ps[:, oo], lhsT=pc_hi_q[qo][HI, pso], rhs=it_hi[HI, xs],
                        start=True, stop=False,
                    )
                    nc.tensor.matmul(
                        o_ps[:, oe], lhsT=pc_hi_q[qe][LO, pse], rhs=it_lo[LO, xs],
                        start=False, stop=False,
                    )
                    nc.tensor.matmul(
                        o_ps[:, oo], lhsT=pc_hi_q[qo][HI, pso], rhs=it_lo[HI, xs],
                        start=False, stop=False,
                    )
                    nc.tensor.matmul(
                        o_ps[:, oe], lhsT=pc_lo_q[qe][LO, pse], rhs=it_hi[LO, xs],
                        start=False, stop=True,
                    )
                    nc.tensor.matmul(
                        o_ps[:, oo], lhsT=pc_lo_q[qo][HI, pso], rhs=it_hi[HI, xs],
                        start=False, stop=True,
                    )
                    # split the copy by columns: DVE is faster than ACT at
                    # f32 PSUM reads in practice, so DVE takes the bigger cut
                    CSPL = 340
                    ot = opool.tile([HKS, 2 * L], dtype=F32)
                    o3 = o_ps[:].rearrange("p (g c) -> p g c", c=L)
                    ot3 = ot[:].rearrange("p (g c) -> p g c", c=L)
                    nc.vector.tensor_copy(
                        out=ot3[:, :, :CSPL], in_=o3[:, :, :CSPL]
                    )
                    nc.scalar.copy(out=ot3[:, :, CSPL:], in_=o3[:, :, CSPL:])
                    # out-DMA on the ACT HWDGE ring: keeps the SP ring free
                    # for input prefetch (no head-of-line blocking)
                    c0 = (ci * CB + jj * 2) * L
                    nc.scalar.dma_start(
                        out=out[:, c0 : c0 + 2 * L], in_=ot[:]
                    )
    nc.finalize()
    return nc


def _host_prep(exchangeability_kernel, equilibrium_kernel):
    """Tiny (H,K,20,20) eigen prep in float64 on host -> BDV, BDW, lam."""
    ek = exchangeability_kernel.astype(np.float64)
    eq = equilibrium_kernel.astype(np.float64)
    Rm = 0.5 * (ek + np.swapaxes(ek, -1, -2))
    Rm = np.logaddexp(0.0, Rm)  # softplus
    Rm = Rm * (1.0 - np.eye(S))
    # softmax
    em = eq - eq.max(axis=-1, keepdims=True)
    p = np.exp(em)
    p /= p.sum(axis=-1, keepdims=True)
    Q = Rm * p[..., None, :]
    row = Q.sum(axis=-1)
    Q = Q - row[..., :, None] * np.eye(S)
    mue = (p * row).sum(axis=-1)[..., None, None]
    Q = Q / np.maximum(mue, 1e-16)
    sqrt_p = np.sqrt(p)
    inv_sqrt_p = 1.0 / sqrt_p
    Sm = sqrt_p[..., :, None] * Q * inv_sqrt_p[..., None, :]
    Sm = 0.5 * (Sm + np.swapaxes(Sm, -1, -2))
    lam, U = np.linalg.eigh(Sm)  # (H,K,S), (H,K,S,S)

    BDV = np.zeros((HZ, HKS), dtype=np.float64)
    BDW = np.zeros((HKS, HKS), dtype=np.float64)
    for h in range(H):
        for k in range(K):
            c = h * K * S + k * S
            # V[z,s] = U[z,s]/sqrt(p[z]) ; rows = (h,z), cols = (h,k,s)
            BDV[h * S : (h + 1) * S, c : c + S] = inv_sqrt_p[h, k][:, None] * U[h, k]
            # BDW[(h,k,s),(h,k,j)] = sqrt(p[j]) * U[j,s]
            BDW[c : c + S, c : c + S] = (sqrt_p[h, k][:, None] * U[h, k]).T
    lam_flat = lam.reshape(HKS)
    return BDV.astype(np.float32), BDW.astype(np.float32), lam_flat.astype(np.float32)


def kernel(inputs, rate_indices, tau_kernel, exchangeability_kernel, equilibrium_kernel):
    inputs = np.asarray(inputs, dtype=np.float32)
    rate_indices = np.asarray(rate_indices)
    tau_kernel = np.asarray(tau_kernel, dtype=np.float32)

    BDV, BDW, lam_flat = _host_prep(
        np.asarray(exchangeability_kernel), np.asarray(equilibrium_kernel)
    )
    BDV_T = np.ascontiguousarray(BDV.T)
    lam_rep = np.broadcast_to(lam_flat, (BPC, HKS)).copy()
    tau_tab = tau_kernel.reshape(H * NUM_RATES, 1)

    if "nc" not in _NC_CACHE:
        _NC_CACHE["nc"] = build_nc()
    nc = _NC_CACHE["nc"]

    in_maps = []
    for c in range(NCORES):
        bsl = slice(c * BPC, (c + 1) * BPC)
        # feature-major stream layout: [40, 65536], bf16 hi/lo split
        inT_c = np.ascontiguousarray(inputs[bsl].reshape(BPC * L, HZ).T)
        hi = inT_c.astype(NPBF16)
        lo = (inT_c - hi.astype(np.float32)).astype(NPBF16)
        hi3 = hi.reshape(HZ, BPC, L)
        lo3 = lo.reshape(HZ, BPC, L)
        hi_e = np.ascontiguousarray(hi3[:, 0::2].reshape(HZ, ROWS // 2))
        hi_o = np.ascontiguousarray(hi3[:, 1::2].reshape(HZ, ROWS // 2))
        lo_e = np.ascontiguousarray(lo3[:, 0::2].reshape(HZ, ROWS // 2))
        lo_o = np.ascontiguousarray(lo3[:, 1::2].reshape(HZ, ROWS // 2))
        offs_c = (
            np.arange(H, dtype=np.int64)[None, :] * NUM_RATES
            + rate_indices[bsl].astype(np.int64)
        ).astype(np.int32)
        in_maps.append(
            {
                "in_hi_e": hi_e,
                "in_hi_o": hi_o,
                "in_lo_e": lo_e,
                "in_lo_o": lo_o,
                "tau_tab": tau_tab,
                "offs": np.ascontiguousarray(offs_c),
                "bdvT": BDV_T,
                "bdw": BDW,
                "lam_rep": lam_rep,
            }
        )

    _NC_CACHE["in_maps"] = in_maps
    res = run_bass_kernel_spmd(nc, in_maps, core_ids=list(range(NCORES)))

    out = np.empty((B, L, H, K, S), dtype=np.float32)
    for c in range(NCORES):
        o = res.results[c]["out"]  # (80, 65536)
        out[c * BPC : (c + 1) * BPC] = o.T.reshape(BPC, L, H, K, S)
    return out
